# revision 1
# baseline (speedup 1.0000x reference)
"""NonLocal block kernel for 8 Trainium2 NeuronCores.

Algebraic restructuring: the softmax-free attention

    s = theta^T phi / N ;  y = s . g^T   (per batch)

is reassociated as y = (G/N) @ theta with G[i,j] = sum_m g[i,m] phi[j,m]
(a [32,32] matrix per batch).  Folding the surrounding 1x1 convs:

    out = (I + W_w (G/N) theta_w) @ target + (W_w (G/N) theta_b + W_b)

so after G is known the whole module is one 64x64 1x1-conv over target.

Sharding: batch b -> core pair (2b, 2b+1); each core of the pair computes
G for its batch redundantly (reads full ref/ref_align for the batch) and
produces half of the spatial output (no cross-core communication).

The G path (conv+pool+G) runs in bf16: G only perturbs the output at the
~1e-3 * 3e-4 level, far below fp32 tolerances.  The final conv over
target stays fp32.

Device layouts (per core):
  refs [128, 16384] bf16 : rows 0:64 = ref[b] (c, h*w), 64:128 = ref_align[b]
  tgt  [128, 4096]  f32  : target half, u-stacked: row u*64+c, col f
  o    [128, 4096]  f32  : output half, same u-stacking
Conv PSUM is 4-way tile_position packed: [128, 512] = (phi|g) x (sliceX|sliceY)
so pooling runs on all 128 partitions.
"""

import sys

for _p in ("/opt/trn_rl_repo",):
    if _p not in sys.path:
        sys.path.insert(0, _p)

import ml_dtypes
import numpy as np

import concourse.bass as bass
import concourse.mybir as mybir
from concourse import bacc
import concourse.tile as tile
from concourse.masks import make_identity
from concourse.bass_utils import run_bass_kernel_spmd

B, C, IC, H, W = 4, 64, 32, 128, 128
N = H * W            # 16384
NH = N // 2          # spatial positions per core (half batch)
M = N // 4           # 4096 pooled positions per batch
FP32 = mybir.dt.float32
BF16 = mybir.dt.bfloat16

_CACHED = {}


def _build_program() -> bass.Bass:
    nc = bacc.Bacc("TRN2", target_bir_lowering=False, debug=False)

    refs = nc.dram_tensor("refs", [128, N], BF16, kind="ExternalInput")
    tgt = nc.dram_tensor("tgt", [128, NH // 2], FP32, kind="ExternalInput")
    pgw = nc.dram_tensor("pgw", [128, IC], BF16, kind="ExternalInput")
    wB = nc.dram_tensor("wB", [IC, 2 * C + 1], FP32, kind="ExternalInput")
    wbr = nc.dram_tensor("wbr", [1, C], FP32, kind="ExternalInput")
    pgbb = nc.dram_tensor("pgbb", [1, 512], BF16, kind="ExternalInput")
    out = nc.dram_tensor("o", [128, NH // 2], FP32, kind="ExternalOutput")

    RCHUNK = 2048         # refs free-dim per DMA (bf16 -> 512 KiB)
    TCHUNK = 1024         # target/out free-dim per DMA (f32 -> 512 KiB)

    with tile.TileContext(nc) as tc:
        with (
            tc.tile_pool(name="const", bufs=1) as cpool,
            tc.tile_pool(name="refsp", bufs=3) as sbR,
            tc.tile_pool(name="small", bufs=2) as sbS,
            tc.tile_pool(name="persist", bufs=1) as pers,
        ):
            # --- constants / weights (small DMAs on the ACT HWDGE queue so
            # they don't delay the refs stream on the SP queue) ---
            def cdma(shape, dt_, src, tag):
                t = cpool.tile(shape, dt_, tag=tag)
                nc.scalar.dma_start(out=t[:], in_=src[:])
                return t

            pgw_sb = cdma([128, IC], BF16, pgw, "pgw")
            wB_sb = cdma([IC, 2 * C + 1], FP32, wB, "wB")
            wbr_sb = cdma([1, C], FP32, wbr, "wbr")
            pgb_sb = cdma([1, 512], BF16, pgbb, "pgb")
            thw_sb = wB_sb[:, 0:C]
            wwT_sb = wB_sb[:, C:2 * C]
            thb_sb = wB_sb[:, 2 * C:2 * C + 1]
            # device-built constants (gpsimd is otherwise idle)
            idb_sb = cpool.tile([128, 128], FP32, tag="identb")
            make_identity(nc, idb_sb[:])
            onesb_sb = cpool.tile([1, 128], BF16, tag="onesb")
            nc.gpsimd.memset(onesb_sb[:], 1.0)
            one_sb = cpool.tile([1, 1], FP32, tag="one")
            nc.gpsimd.memset(one_sb[:], 1.0)

            # pooled conv outputs (f32), 4-way stacked to 128 partitions
            pooled = pers.tile([128, 16 * 128], FP32, tag="pooled")
            # transposed pooled (+bias), bf16
            phigT = pers.tile([128, 16 * 128], BF16, tag="phigT")

            tgt_tiles = []
            for tq in range((NH // 2) // TCHUNK):
                tgt_tile = pers.tile([128, TCHUNK], FP32, tag=f"tgt{tq}")
                tgt_tiles.append(tgt_tile)

            # ---- Phase A: convs + fused 2x2 maxpool, with transpose/G/W4
            # accumulation streamed per 4-chunk group ----
            with tc.tile_pool(name="psA", bufs=4, space="PSUM") as psA, \
                 tc.tile_pool(name="psB", bufs=1, space="PSUM") as psB, \
                 tc.tile_pool(name="psG", bufs=1, space="PSUM") as psG, \
                 tc.tile_pool(name="psW", bufs=1, space="PSUM") as psW:
                w4_ps = psW.tile([128, C], FP32, tag="w4")
                v_ps = psW.tile([IC, 1], FP32, tag="v")
                for cpos in (0, 64):
                    nc.tensor.matmul(
                        w4_ps[cpos:cpos + C, :], idb_sb[0:C, 0:C],
                        idb_sb[0:C, 0:C], start=True, stop=False,
                        tile_position=(0, cpos), skip_group_check=True,
                    )
                tpp_box = [None]

                def emit_transpose(blk):
                    if blk % 4 == 0:
                        tpp_new = psB.tile([128, 512], FP32, tag="tp")
                        tpp_box[0] = tpp_new
                        nc.tensor.matmul(
                            tpp_box[0][:], onesb_sb[:], pgb_sb[:],
                            start=True, stop=False, skip_group_check=True,
                        )
                    bi = blk % 4
                    nc.tensor.matmul(
                        tpp_box[0][:, 128 * bi:128 * (bi + 1)],
                        pooled[:, 128 * blk:128 * (blk + 1)],
                        idb_sb[:],
                        is_transpose=True, start=False, stop=True,
                        skip_group_check=True,
                    )

                def emit_group_tail(t):
                    # bias'd transposed group -> bf16, fold into G, then
                    # stream the partial-W4 chain
                    nc.scalar.activation(
                        phigT[:, 512 * t:512 * (t + 1)], tpp_box[0][:],
                        mybir.ActivationFunctionType.Copy,
                    )
                    g_ps = psG.tile([IC, IC], FP32, tag="G")
                    for c in range(4 * t, 4 * t + 4):
                        b0 = 128 * c
                        nc.tensor.matmul(
                            g_ps[:], phigT[:, b0:b0 + IC],
                            phigT[:, b0 + IC:b0 + 2 * IC],
                            start=(c % 4 == 0), stop=False,
                            skip_group_check=True,
                        )
                        nc.tensor.matmul(
                            g_ps[:], phigT[:, b0 + 2 * IC:b0 + 3 * IC],
                            phigT[:, b0 + 3 * IC:b0 + 4 * IC],
                            start=False, stop=(c % 4 == 3),
                            skip_group_check=True,
                        )
                    gt_sb = sbS.tile([IC, IC], FP32, tag="Gt")
                    nc.scalar.activation(
                        gt_sb[:], g_ps[:],
                        mybir.ActivationFunctionType.Copy, scale=1.0 / N,
                    )
                    nc.tensor.matmul(v_ps[:], gt_sb[:], thb_sb,
                                     start=(t == 0), stop=(t == 3),
                                     skip_group_check=True)
                    m2_ps = psG.tile([IC, C], FP32, tag="G")
                    nc.tensor.matmul(m2_ps[:], gt_sb[:], thw_sb,
                                     start=True, stop=True,
                                     skip_group_check=True)
                    m2_sb = sbS.tile([IC, C], FP32, tag="m2sb")
                    nc.scalar.activation(
                        m2_sb[:], m2_ps[:], mybir.ActivationFunctionType.Copy
                    )
                    for cpos in (0, 64):
                        nc.tensor.matmul(
                            w4_ps[cpos:cpos + C, :], m2_sb[:], wwT_sb,
                            start=False, stop=(t == 3 and cpos == 64),
                            tile_position=(0, cpos),
                            skip_group_check=True,
                        )

                for k in range(N // RCHUNK):
                    rt = sbR.tile([128, RCHUNK], BF16, tag="refs")
                    nc.sync.dma_start(
                        out=rt[:], in_=refs[:, k * RCHUNK:(k + 1) * RCHUNK]
                    )
                    for j in range(RCHUNK // 1024):
                        cidx = k * (RCHUNK // 1024) + j  # 0..15
                        t = cidx // 4
                        xs = slice(j * 1024, j * 1024 + 512)
                        ys = slice(j * 1024 + 512, (j + 1) * 1024)
                        cp = psA.tile([128, 512], FP32, tag="conv")
                        nc.tensor.matmul(cp[0:32, :], pgw_sb[0:C, :],
                                         rt[0:C, xs], start=True, stop=True,
                                         tile_position=(0, 0))
                        nc.tensor.matmul(cp[32:64, :], pgw_sb[C:128, :],
                                         rt[C:128, xs], start=True, stop=True,
                                         tile_position=(64, 32))
                        nc.tensor.matmul(cp[64:96, :], pgw_sb[0:C, :],
                                         rt[0:C, ys], start=True, stop=True,
                                         tile_position=(0, 64))
                        nc.tensor.matmul(cp[96:128, :], pgw_sb[C:128, :],
                                         rt[C:128, ys], start=True, stop=True,
                                         tile_position=(64, 96))
                        # fused 2x2 maxpool: windowed reduce over (h0, w0)
                        po = pooled[:, cidx * 128:(cidx + 1) * 128]
                        nc.vector.reduce_max(
                            po.rearrange("p (hp w) -> p hp w", w=W // 2),
                            cp[:].rearrange("p (hp h0 w w0) -> p hp w h0 w0",
                                            hp=2, h0=2, w=W // 2, w0=2),
                            axis=mybir.AxisListType.XY,
                        )
                        # transpose pipeline runs one 4-chunk group behind
                        prev = cidx - 4
                        if prev >= 0:
                            emit_transpose(prev)
                            if prev % 4 == 3:
                                emit_group_tail(prev // 4)
                        if cidx == 7:
                            # mid-phase: prefetch target on the ACT queue
                            for tq in range((NH // 2) // TCHUNK):
                                nc.scalar.dma_start(
                                    out=tgt_tiles[tq][:],
                                    in_=tgt[:, tq * TCHUNK:(tq + 1) * TCHUNK],
                                )

                for prev in range(12, 16):
                    emit_transpose(prev)
                emit_group_tail(3)

                w4_sb = pers.tile([128, C], FP32, tag="w4sb")
                nc.scalar.activation(
                    w4_sb[:], w4_ps[:], mybir.ActivationFunctionType.Copy
                )
                v_sb = pers.tile([IC, 1], FP32, tag="vsb")
                nc.scalar.activation(
                    v_sb[:], v_ps[:], mybir.ActivationFunctionType.Copy
                )
                # b2 as a per-partition column, duplicated on partitions 64:128
                b2c_ps = psG.tile([128, 1], FP32, tag="G")
                for cpos in (0, 64):
                    nc.tensor.matmul(
                        b2c_ps[cpos:cpos + C, :], wwT_sb, v_sb[:],
                        start=True, stop=False, tile_position=(0, cpos),
                        skip_group_check=True,
                    )
                    nc.tensor.matmul(
                        b2c_ps[cpos:cpos + C, :], wbr_sb[:], one_sb[:, :],
                        start=False, stop=True, tile_position=(0, cpos),
                        skip_group_check=True,
                    )
                b2c_sb = pers.tile([128, 1], FP32, tag="b2csb")
                nc.scalar.activation(
                    b2c_sb[:], b2c_ps[:], mybir.ActivationFunctionType.Copy
                )

            # ---------- Phase D: final 64x64 conv over target (fp32) ----------
            with tc.tile_pool(name="psD", bufs=3, space="PSUM") as psD, \
                 tc.tile_pool(name="outp", bufs=2) as sbO:
                for t in range((NH // 2) // TCHUNK):
                    tt = tgt_tiles[t]
                    ot = sbO.tile([128, TCHUNK], FP32, tag="out")
                    for i in range(TCHUNK // 512):
                        op = psD.tile([128, 512], FP32, tag="od")
                        isl = slice(i * 512, (i + 1) * 512)
                        nc.tensor.matmul(
                            op[0:C, :], w4_sb[0:C, :], tt[0:C, isl],
                            start=True, stop=True, tile_position=(0, 0),
                        )
                        nc.tensor.matmul(
                            op[C:128, :], w4_sb[C:128, :], tt[C:128, isl],
                            start=True, stop=True, tile_position=(64, 64),
                        )
                        nc.scalar.activation(
                            ot[:, isl], op[:],
                            mybir.ActivationFunctionType.Identity,
                            bias=b2c_sb[:],
                        )
                    nc.sync.dma_start(
                        out=out[:, t * TCHUNK:(t + 1) * TCHUNK], in_=ot[:]
                    )

    nc.compile()
    return nc


def _in_maps(target, ref, ref_align, theta_w, theta_b, phi_w, phi_b,
             g_w, g_b, W_w, W_b):
    f32, bf16 = np.float32, ml_dtypes.bfloat16
    wBv = np.zeros((IC, 2 * C + 1), dtype=f32)
    wBv[:, 0:C] = theta_w
    wBv[:, C:2 * C] = W_w.T
    wBv[:, 2 * C] = theta_b
    common = {
        "pgw": np.concatenate([phi_w.T, g_w.T], axis=0).astype(bf16),
        "wB": wBv,
        "wbr": W_b.reshape(1, C).astype(f32),
        "pgbb": np.tile(np.concatenate([phi_b, g_b]), 8).reshape(1, 512).astype(bf16),
    }
    maps = []
    for core in range(8):
        b, u = core // 2, core % 2
        refs = np.concatenate(
            [ref[b].reshape(C, N), ref_align[b].reshape(C, N)], axis=0
        ).astype(bf16)
        th = target[b, :, u * (H // 2):(u + 1) * (H // 2), :].reshape(C, NH)
        tgtv = np.concatenate([th[:, :NH // 2], th[:, NH // 2:]], axis=0).astype(f32)
        maps.append({"refs": np.ascontiguousarray(refs),
                     "tgt": np.ascontiguousarray(tgtv), **common})
    return maps


def kernel(**inputs) -> np.ndarray:
    if "nc" not in _CACHED:
        _CACHED["nc"] = _build_program()
    nc = _CACHED["nc"]
    maps = _in_maps(**inputs)
    res = run_bass_kernel_spmd(nc, maps, list(range(8)))
    out = np.empty((B, C, H, W), dtype=np.float32)
    for core in range(8):
        o = res.results[core]["o"]  # [128, 4096] u-stacked
        half = np.concatenate([o[:C, :], o[C:, :]], axis=1)  # [64, 8192]
        b, u = core // 2, core % 2
        out[b, :, u * (H // 2):(u + 1) * (H // 2), :] = half.reshape(C, H // 2, W)
    return out



# revision 12
# speedup vs baseline: 1.3201x; 1.3201x over previous
"""NonLocal block kernel for 8 Trainium2 NeuronCores.

Algebraic restructuring: the softmax-free attention

    s = theta^T phi / N ;  y = s . g^T   (per batch)

is reassociated as y = (G/N) @ theta with G[i,j] = sum_m g[i,m] phi[j,m]
(a [32,32] matrix per batch).  Folding the surrounding 1x1 convs:

    out = (I + W_w (G/N) theta_w) @ target + (W_w (G/N) theta_b + W_b)

so after G is known the whole module is one 64x64 1x1-conv over target.

Sharding: batch b -> core pair (2b, 2b+1); each core of the pair computes
G for its batch redundantly (reads full ref/ref_align for the batch) and
produces half of the spatial output (no cross-core communication).

Precision: the G path only perturbs the output at the ~1e-3 * 3e-4 level,
far below the fp32 tolerance, so refs stream in fp8 (e3m4) and the
phi/g conv runs in fp8.  target / output are bf16 (the final conv
accumulates in fp32 PSUM); worst-case output error ~0.5% vs 2% budget.

Device layouts (per core):
  refs [128, 16384] f8e3 : rows 0:64 = ref[b] (c, h*w), 64:128 = ref_align[b]
  tgt  [128, 4096]  bf16 : target half, u-stacked: partitions 0:64 = first
                           2048 cols of the (c, 64*128) half, 64:128 = rest
  o    [128, 4096]  bf16 : output half, same u-stacking
Conv weights are block-diagonal [128 -> 64]: psum partitions 0:32 = phi,
32:64 = g for slice A; a second copy at PE column-group 64 computes the
next 512 positions concurrently (partitions 64:128), so the two matmuls
per 1024 positions stream in parallel on separate XBUSes.
"""

import sys

for _p in ("/opt/trn_rl_repo",):
    if _p not in sys.path:
        sys.path.insert(0, _p)

import ml_dtypes
import numpy as np

import concourse.bass as bass
import concourse.mybir as mybir
from concourse import bacc
import concourse.tile as tile
from concourse.masks import make_identity
from concourse.bass_utils import run_bass_kernel_spmd

B, C, IC, H, W = 4, 64, 32, 128, 128
N = H * W            # 16384 positions per batch
NT = N // 4          # 4096 columns of u-stacked target half per core
FP32 = mybir.dt.float32
BF16 = mybir.dt.bfloat16
F8 = mybir.dt.float8e3

RCHUNK = 4096        # refs cols per DMA (fp8 -> 512 KiB)

_CACHED = {}


def _build_program() -> bass.Bass:
    nc = bacc.Bacc("TRN2", target_bir_lowering=False, debug=False)

    refs = nc.dram_tensor("refs", [128, N], F8, kind="ExternalInput")
    tgt = nc.dram_tensor("tgt", [128, NT], BF16, kind="ExternalInput")
    wbd = nc.dram_tensor("wbd", [128, C], F8, kind="ExternalInput")
    wB = nc.dram_tensor("wB", [IC, 2 * C + 1], FP32, kind="ExternalInput")
    wbr = nc.dram_tensor("wbr", [1, C], FP32, kind="ExternalInput")
    pgbf = nc.dram_tensor("pgbf", [128, 512], BF16, kind="ExternalInput")
    pgbc = nc.dram_tensor("pgbc", [128, 1], FP32, kind="ExternalInput")
    out = nc.dram_tensor("o", [128, NT], BF16, kind="ExternalOutput")

    AF = mybir.ActivationFunctionType
    ALU = mybir.AluOpType

    with tile.TileContext(nc) as tc:
        with (
            tc.tile_pool(name="const", bufs=1) as cpool,
            tc.tile_pool(name="refsp", bufs=3) as sbR,
            tc.tile_pool(name="small", bufs=2) as sbS,
            tc.tile_pool(name="outp", bufs=2) as sbO,
            tc.tile_pool(name="persist", bufs=1) as pers,
        ):
            # --- small weights first on the single sync HWDGE queue ---
            wbd_sb = cpool.tile([128, C], F8, tag="wbd")
            nc.sync.dma_start(out=wbd_sb[:], in_=wbd[:])
            wB_sb = cpool.tile([IC, 2 * C + 1], FP32, tag="wB")
            nc.sync.dma_start(out=wB_sb[:], in_=wB[:])
            wbr_sb = cpool.tile([1, C], FP32, tag="wbr")
            nc.sync.dma_start(out=wbr_sb[:], in_=wbr[:])
            pgb_sb = cpool.tile([128, 512], BF16, tag="pgb")
            nc.sync.dma_start(out=pgb_sb[:], in_=pgbf[:])
            pgbc_sb = cpool.tile([128, 1], FP32, tag="pgbc")
            nc.sync.dma_start(out=pgbc_sb[:], in_=pgbc[:])
            thw_sb = wB_sb[:, 0:C]
            wwT_sb = wB_sb[:, C:2 * C]
            thb_sb = wB_sb[:, 2 * C:2 * C + 1]

            # device-built constants (gpsimd is otherwise idle at start)
            idb_sb = cpool.tile([128, 128], BF16, tag="identb")
            make_identity(nc, idb_sb[:])
            one_sb = cpool.tile([1, 1], FP32, tag="one")
            nc.gpsimd.memset(one_sb[:], 1.0)

            pooled = pers.tile([128, 2048], BF16, tag="pooled")
            phigT = pers.tile([128, 2048], BF16, tag="phigT")
            tgt_tiles = [pers.tile([128, 1024], BF16, tag=f"tgt{i}",
                                   name=f"tgt{i}") for i in range(4)]

            with tc.tile_pool(name="psA", bufs=3, space="PSUM") as psA, \
                 tc.tile_pool(name="psB", bufs=2, space="PSUM") as psB, \
                 tc.tile_pool(name="psG", bufs=1, space="PSUM") as psG, \
                 tc.tile_pool(name="psW", bufs=1, space="PSUM") as psW:
                g_ps = psG.tile([IC, IC], FP32, tag="G")
                w4_ps = psW.tile([128, C], FP32, tag="w4")
                # identity seed of the fused final-conv weight (I + A^T),
                # duplicated for both u-halves; accumulation closes after G.
                for cpos in (0, 64):
                    nc.tensor.matmul(
                        w4_ps[cpos:cpos + C, :], idb_sb[0:C, 0:C],
                        idb_sb[0:C, 0:C], start=True, stop=False,
                        tile_position=(0, cpos), skip_group_check=True,
                    )

                def emit_gwork(k):
                    # transposes + pooled-bias + G partials for refs chunk k
                    tp = psB.tile([128, 512], BF16, tag="tp")
                    for j in range(4):
                        cidx = 4 * k + j
                        nc.tensor.matmul(
                            tp[:, j * 128:(j + 1) * 128],
                            pooled[:, cidx * 128:(cidx + 1) * 128],
                            idb_sb[:], is_transpose=True,
                            start=True, stop=True, skip_group_check=True,
                        )
                    if k == 0:
                        # chunk 0 pooled pre-bias: add phi/g biases here
                        nc.vector.scalar_tensor_tensor(
                            out=phigT[:, k * 512:(k + 1) * 512], in0=tp[:],
                            scalar=1.0, in1=pgb_sb[:],
                            op0=ALU.mult, op1=ALU.add,
                        )
                    else:
                        # bias was folded into the ACT psum->bf16 conv copy
                        nc.scalar.activation(
                            phigT[:, k * 512:(k + 1) * 512], tp[:], AF.Copy,
                        )
                    for j in range(4):
                        b0 = k * 512 + j * 128
                        nc.tensor.matmul(
                            g_ps[:], phigT[:, b0:b0 + IC],
                            phigT[:, b0 + IC:b0 + 2 * IC],
                            start=(k == 0 and j == 0), stop=False,
                            skip_group_check=True,
                        )
                        nc.tensor.matmul(
                            g_ps[:], phigT[:, b0 + 2 * IC:b0 + 3 * IC],
                            phigT[:, b0 + 3 * IC:b0 + 4 * IC],
                            start=False, stop=(k == 3 and j == 3),
                            skip_group_check=True,
                        )

                # ---- Phase A: fp8 conv + fused 2x2 maxpool over refs ----
                for k in range(N // RCHUNK):
                    rt = sbR.tile([128, RCHUNK], F8, tag="refs")
                    nc.sync.dma_start(
                        out=rt[:], in_=refs[:, k * RCHUNK:(k + 1) * RCHUNK]
                    )
                    for j in range(RCHUNK // 1024):
                        cidx = 4 * k + j
                        xs = slice(j * 1024, j * 1024 + 512)
                        ys = slice(j * 1024 + 512, (j + 1) * 1024)
                        cp = psA.tile([128, 512], FP32, tag="conv")
                        nc.tensor.matmul(cp[0:C, :], wbd_sb[:], rt[:, xs],
                                         start=True, stop=True,
                                         tile_position=(0, 0))
                        nc.tensor.matmul(cp[C:128, :], wbd_sb[:], rt[:, ys],
                                         start=True, stop=True,
                                         tile_position=(0, 64))
                        # fused 2x2 maxpool
                        po = pooled[:, cidx * 128:(cidx + 1) * 128]
                        if k == 0:
                            # direct windowed reduce from PSUM (DVE, f32)
                            nc.vector.reduce_max(
                                po.rearrange("p (hp w) -> p hp w", w=W // 2),
                                cp[:].rearrange(
                                    "p (hp h0 w w0) -> p hp w h0 w0",
                                    hp=2, h0=2, w=W // 2, w0=2),
                                axis=mybir.AxisListType.XY,
                            )
                        else:
                            # ACT: psum -> bf16 with fused conv bias, then
                            # DVE pools in bf16 at 2x rate
                            cb = sbS.tile([128, 512], BF16, tag="cb")
                            nc.scalar.activation(
                                cb[:], cp[:], AF.Identity, bias=pgbc_sb[:],
                            )
                            s1 = sbS.tile([128, 256], BF16, tag="s1")
                            nc.vector.reduce_max(
                                s1[:],
                                cb[:].rearrange("p (a w0) -> p a w0", w0=2),
                                axis=mybir.AxisListType.X,
                            )
                            s1r = s1.rearrange("p (hp h0 w) -> p hp h0 w",
                                               hp=2, h0=2, w=W // 2)
                            nc.vector.tensor_max(
                                po.rearrange("p (hp o w) -> p hp o w",
                                             o=1, w=W // 2),
                                s1r[:, :, 0:1, :], s1r[:, :, 1:2, :],
                            )
                    if k == N // RCHUNK - 1:
                        # target stream queues behind the last refs chunk
                        for t in range(4):
                            nc.sync.dma_start(
                                out=tgt_tiles[t][:],
                                in_=tgt[:, t * 1024:(t + 1) * 1024],
                            )
                    if k >= 1:
                        emit_gwork(k - 1)
                emit_gwork(N // RCHUNK - 1)

                # ---- G chain: fold G into the 64x64 conv + bias column ----
                gt_sb = sbS.tile([IC, IC], FP32, tag="Gt")
                nc.scalar.activation(gt_sb[:], g_ps[:], AF.Copy, scale=1.0 / N)
                v_ps = psW.tile([IC, 1], FP32, tag="v")
                nc.tensor.matmul(v_ps[:], gt_sb[:], thb_sb,
                                 start=True, stop=True, skip_group_check=True)
                m2_ps = psG.tile([IC, C], FP32, tag="G")
                nc.tensor.matmul(m2_ps[:], gt_sb[:], thw_sb,
                                 start=True, stop=True, skip_group_check=True)
                m2_sb = sbS.tile([IC, C], FP32, tag="m2sb")
                nc.scalar.activation(m2_sb[:], m2_ps[:], AF.Copy)
                for cpos in (0, 64):
                    nc.tensor.matmul(
                        w4_ps[cpos:cpos + C, :], m2_sb[:], wwT_sb,
                        start=False, stop=(cpos == 64),
                        tile_position=(0, cpos), skip_group_check=True,
                    )
                w4_sb = pers.tile([128, C], BF16, tag="w4sb")
                nc.scalar.activation(w4_sb[:], w4_ps[:], AF.Copy)
                v_sb = sbS.tile([IC, 1], FP32, tag="vsb")
                nc.scalar.activation(v_sb[:], v_ps[:], AF.Copy)
                b2c_ps = psG.tile([128, 1], FP32, tag="G")
                for cpos in (0, 64):
                    nc.tensor.matmul(
                        b2c_ps[cpos:cpos + C, :], wwT_sb, v_sb[:],
                        start=True, stop=False, tile_position=(0, cpos),
                        skip_group_check=True,
                    )
                    nc.tensor.matmul(
                        b2c_ps[cpos:cpos + C, :], wbr_sb[:], one_sb[:, :],
                        start=False, stop=True, tile_position=(0, cpos),
                        skip_group_check=True,
                    )
                b2c_sb = pers.tile([128, 1], FP32, tag="b2csb")
                nc.scalar.activation(b2c_sb[:], b2c_ps[:], AF.Copy)

            # ---- Phase D: final 64x64 conv over target (bf16) ----
            with tc.tile_pool(name="psD", bufs=3, space="PSUM") as psD:
                for t in range(4):
                    tt = tgt_tiles[t]
                    ot = sbO.tile([128, 1024], BF16, tag="out")
                    for i in range(2):
                        op = psD.tile([128, 512], FP32, tag="od")
                        isl = slice(i * 512, (i + 1) * 512)
                        nc.tensor.matmul(
                            op[0:C, :], w4_sb[0:C, :], tt[0:C, isl],
                            start=True, stop=True, tile_position=(0, 0),
                        )
                        nc.tensor.matmul(
                            op[C:128, :], w4_sb[C:128, :], tt[C:128, isl],
                            start=True, stop=True, tile_position=(64, 64),
                        )
                        if (2 * t + i) % 2 == 0:
                            nc.scalar.activation(
                                ot[:, isl], op[:], AF.Identity,
                                bias=b2c_sb[:],
                            )
                        else:
                            nc.vector.tensor_scalar_add(
                                ot[:, isl], op[:], b2c_sb[:],
                            )
                    nc.sync.dma_start(
                        out=out[:, t * 1024:(t + 1) * 1024], in_=ot[:]
                    )

    nc.compile()
    return nc


def _in_maps(target, ref, ref_align, theta_w, theta_b, phi_w, phi_b,
             g_w, g_b, W_w, W_b):
    f32 = np.float32
    bf16 = ml_dtypes.bfloat16
    f8 = ml_dtypes.float8_e3m4
    wBv = np.zeros((IC, 2 * C + 1), dtype=f32)
    wBv[:, 0:C] = theta_w
    wBv[:, C:2 * C] = W_w.T
    wBv[:, 2 * C] = theta_b
    wbdv = np.zeros((128, C), dtype=f32)
    wbdv[0:C, 0:IC] = phi_w.T
    wbdv[C:128, IC:2 * IC] = g_w.T
    pgbv = np.broadcast_to(
        np.tile(np.concatenate([phi_b, g_b]), 8)[None, :], (128, 512)
    )
    pgbcv = np.tile(np.concatenate([phi_b, g_b]), 2).reshape(128, 1)
    common = {
        "wbd": wbdv.astype(f8),
        "wB": wBv,
        "wbr": W_b.reshape(1, C).astype(f32),
        "pgbf": np.ascontiguousarray(pgbv).astype(bf16),
        "pgbc": pgbcv.astype(f32),
    }
    maps = []
    for core in range(8):
        b, u = core // 2, core % 2
        refsv = np.concatenate(
            [ref[b].reshape(C, N), ref_align[b].reshape(C, N)], axis=0
        ).astype(f8)
        th = target[b, :, u * (H // 2):(u + 1) * (H // 2), :].reshape(C, N // 2)
        tgtv = np.concatenate([th[:, :NT], th[:, NT:]], axis=0).astype(bf16)
        maps.append({"refs": np.ascontiguousarray(refsv),
                     "tgt": np.ascontiguousarray(tgtv), **common})
    return maps


def kernel(**inputs) -> np.ndarray:
    if "nc" not in _CACHED:
        _CACHED["nc"] = _build_program()
    nc = _CACHED["nc"]
    maps = _in_maps(**inputs)
    res = run_bass_kernel_spmd(nc, maps, list(range(8)))
    out = np.empty((B, C, H, W), dtype=np.float32)
    for core in range(8):
        o = res.results[core]["o"].astype(np.float32)  # [128, 4096] u-stacked
        half = np.concatenate([o[:C, :], o[C:, :]], axis=1)  # [64, 8192]
        b, u = core // 2, core % 2
        out[b, :, u * (H // 2):(u + 1) * (H // 2), :] = half.reshape(C, H // 2, W)
    return out


# revision 17
# speedup vs baseline: 1.3267x; 1.0050x over previous
"""NonLocal block kernel for 8 Trainium2 NeuronCores.

Algebraic restructuring: the softmax-free attention

    s = theta^T phi / N ;  y = s . g^T   (per batch)

is reassociated as y = (G/N) @ theta with G[i,j] = sum_m g[i,m] phi[j,m]
(a [32,32] matrix per batch).  Folding the surrounding 1x1 convs:

    out = (I + W_w (G/N) theta_w) @ target + (W_w (G/N) theta_b + W_b)

so after G is known the whole module is one 64x64 1x1-conv over target.

Sharding: batch b -> core pair (2b, 2b+1); each core of the pair computes
G for its batch redundantly (reads full ref/ref_align for the batch) and
produces half of the spatial output (no cross-core communication).

Precision: the G path only perturbs the output at the ~1e-3 * 3e-4 level,
far below the fp32 tolerance, so refs stream in fp8 (e3m4) and the
phi/g conv runs in fp8.  target / output are bf16 (the final conv
accumulates in fp32 PSUM); worst-case output error ~0.7% vs 2% budget.

Device layouts (per core):
  refs [128, 16384] f8e3 : rows 0:64 = ref[b] (c, h*w), 64:128 = ref_align[b]
  tgt  [128, 4096]  bf16 : target half, u-stacked: partitions 0:64 = first
                           2048 cols of the (c, 64*128) half, 64:128 = rest
  o    [128, 4096]  bf16 : output half, same u-stacking
Conv weights are block-diagonal [128 -> 64] (psum partitions 0:32 = phi,
32:64 = g); a second copy at PE column-group 64 computes the next 512
positions concurrently (partitions 64:128), so matmul pairs stream in
parallel on separate XBUSes.  2x2 maxpool = two DVE tensor_max stages
(w-pairs from PSUM, then h-pairs in bf16).  phi/g biases enter via a
PE outer-product seed of the transpose PSUM tile.
"""

import sys

for _p in ("/opt/trn_rl_repo",):
    if _p not in sys.path:
        sys.path.insert(0, _p)

import ml_dtypes
import numpy as np

import concourse.bass as bass
import concourse.mybir as mybir
from concourse import bacc
import concourse.tile as tile
from concourse.masks import make_identity
from concourse.bass_utils import run_bass_kernel_spmd

B, C, IC, H, W = 4, 64, 32, 128, 128
N = H * W            # 16384 positions per batch
NT = N // 4          # 4096 columns of u-stacked target half per core
FP32 = mybir.dt.float32
BF16 = mybir.dt.bfloat16
F8 = mybir.dt.float8e3

# refs DMA chunks (fp8 cols): small first chunks so the conv starts early
RCHUNKS = [2048, 2048, 4096, 4096, 4096]

_CACHED = {}


def _build_program() -> bass.Bass:
    nc = bacc.Bacc("TRN2", target_bir_lowering=False, debug=False)

    refs = nc.dram_tensor("refs", [128, N], F8, kind="ExternalInput")
    tgt = nc.dram_tensor("tgt", [128, NT], BF16, kind="ExternalInput")
    wbd = nc.dram_tensor("wbd", [128, C], F8, kind="ExternalInput")
    wB = nc.dram_tensor("wB", [IC, 2 * C + 1], FP32, kind="ExternalInput")
    wbr = nc.dram_tensor("wbr", [1, C], FP32, kind="ExternalInput")
    pgbf = nc.dram_tensor("pgbf", [1, 512], BF16, kind="ExternalInput")
    out = nc.dram_tensor("o", [128, NT], BF16, kind="ExternalOutput")

    AF = mybir.ActivationFunctionType

    with tile.TileContext(nc) as tc:
        with (
            tc.tile_pool(name="const", bufs=1) as cpool,
            tc.tile_pool(name="refsp", bufs=3) as sbR,
            tc.tile_pool(name="small", bufs=2) as sbS,
            tc.tile_pool(name="outp", bufs=2) as sbO,
            tc.tile_pool(name="persist", bufs=1) as pers,
        ):
            # refs chunk 0 goes first on the sync HWDGE queue; all small
            # weights ride the scalar HWDGE queue concurrently.
            rts = []
            col = 0
            for k, rc in enumerate(RCHUNKS):
                rt = sbR.tile([128, rc], F8, tag=f"refs{min(k, 2)}",
                              name=f"refs{min(k, 2)}")
                nc.sync.dma_start(out=rt[:], in_=refs[:, col:col + rc])
                rts.append(rt)
                col += rc
            tgt_tiles = []
            for t in range(2):
                tt = pers.tile([128, 2048], BF16, tag=f"tgt{t}",
                               name=f"tgt{t}")
                nc.sync.dma_start(
                    out=tt[:], in_=tgt[:, t * 2048:(t + 1) * 2048])
                tgt_tiles.append(tt)

            wbd_sb = cpool.tile([128, C], F8, tag="wbd")
            nc.scalar.dma_start(out=wbd_sb[:], in_=wbd[:])
            wB_sb = cpool.tile([IC, 2 * C + 1], FP32, tag="wB")
            nc.scalar.dma_start(out=wB_sb[:], in_=wB[:])
            wbr_sb = cpool.tile([1, C], FP32, tag="wbr")
            nc.scalar.dma_start(out=wbr_sb[:], in_=wbr[:])
            pgb_sb = cpool.tile([1, 512], BF16, tag="pgb")
            nc.scalar.dma_start(out=pgb_sb[:], in_=pgbf[:])
            thw_sb = wB_sb[:, 0:C]
            wwT_sb = wB_sb[:, C:2 * C]
            thb_sb = wB_sb[:, 2 * C:2 * C + 1]

            # device-built constants (gpsimd is otherwise idle)
            idb_sb = cpool.tile([128, 128], BF16, tag="identb")
            make_identity(nc, idb_sb[:])
            idf_sb = cpool.tile([128, 128], FP32, tag="identf")
            make_identity(nc, idf_sb[:])
            onesb_sb = cpool.tile([1, 128], BF16, tag="onesb")
            nc.gpsimd.memset(onesb_sb[:], 1.0)
            one_sb = cpool.tile([1, 1], FP32, tag="one")
            nc.gpsimd.memset(one_sb[:], 1.0)

            pooled = pers.tile([128, 2048], FP32, tag="pooled")
            phigT = pers.tile([128, 2048], BF16, tag="phigT")

            with tc.tile_pool(name="psA", bufs=2, space="PSUM") as psA, \
                 tc.tile_pool(name="psB", bufs=2, space="PSUM") as psB, \
                 tc.tile_pool(name="psG", bufs=1, space="PSUM") as psG, \
                 tc.tile_pool(name="psW", bufs=1, space="PSUM") as psW:
                g_ps = psG.tile([IC, IC], FP32, tag="G")
                w4_ps = psW.tile([128, C], FP32, tag="w4")
                # identity seed of the fused final-conv weight (I + A^T),
                # duplicated for both u-halves; group closes after G.
                for cpos in (0, 64):
                    nc.tensor.matmul(
                        w4_ps[cpos:cpos + C, :], idb_sb[0:C, 0:C],
                        idb_sb[0:C, 0:C], start=True, stop=False,
                        tile_position=(0, cpos), skip_group_check=True,
                    )

                tp_tiles = {}

                def emit_seed(t):
                    # bias seed: tp = ones(128) x pgb  (phi|g biases tiled)
                    tp = psB.tile([128, 512], FP32, tag="tp")
                    tp_tiles[t] = tp
                    nc.tensor.matmul(tp[:], onesb_sb[:], pgb_sb[:],
                                     start=True, stop=False,
                                     skip_group_check=True)

                # early PE work: warms the HAM clock gate during refs DMA
                emit_seed(0)
                emit_seed(1)

                def emit_gwork(t):
                    # transposes + psum->sbuf + G partials for tp tile t
                    # (pooled blocks 4t .. 4t+3)
                    if t not in tp_tiles:
                        emit_seed(t)
                    tp = tp_tiles.pop(t)
                    for j in range(4):
                        cidx = 4 * t + j
                        nc.tensor.matmul(
                            tp[:, j * 128:(j + 1) * 128],
                            pooled[:, cidx * 128:(cidx + 1) * 128],
                            idf_sb[:], is_transpose=True,
                            start=False, stop=True, skip_group_check=True,
                        )
                    nc.scalar.activation(
                        phigT[:, t * 512:(t + 1) * 512], tp[:], AF.Copy)
                    for j in range(4):
                        b0 = t * 512 + j * 128
                        nc.tensor.matmul(
                            g_ps[:], phigT[:, b0:b0 + IC],
                            phigT[:, b0 + IC:b0 + 2 * IC],
                            start=(t == 0 and j == 0), stop=False,
                            skip_group_check=True,
                        )
                        nc.tensor.matmul(
                            g_ps[:], phigT[:, b0 + 2 * IC:b0 + 3 * IC],
                            phigT[:, b0 + 3 * IC:b0 + 4 * IC],
                            start=False, stop=(t == 3 and j == 3),
                            skip_group_check=True,
                        )

                # ---- Phase A: fp8 conv + fused 2x2 maxpool over refs ----
                g = 0  # 2048-position group counter (0..7)
                for k, rc in enumerate(RCHUNKS):
                    rt = rts[k]
                    for j in range(rc // 2048):
                        cp = psA.tile([128, 1024], FP32, tag="conv")
                        for h in range(2):  # two 512-col slices per group
                            sl = slice(j * 2048 + h * 1024,
                                       j * 2048 + h * 1024 + 512)
                            s2 = slice(j * 2048 + h * 1024 + 512,
                                       j * 2048 + (h + 1) * 1024)
                            nc.tensor.matmul(
                                cp[0:C, h * 512:(h + 1) * 512],
                                wbd_sb[:], rt[:, sl],
                                start=True, stop=True, tile_position=(0, 0))
                            nc.tensor.matmul(
                                cp[C:128, h * 512:(h + 1) * 512],
                                wbd_sb[:], rt[:, s2],
                                start=True, stop=True, tile_position=(0, 64))
                        # 2x2 maxpool, alternating DVE-direct / ACT+DVE
                        po = pooled[:, g * 256:(g + 1) * 256]
                        if g % 2 == 0:
                            nc.vector.reduce_max(
                                po.rearrange("p (s hp w) -> p s hp w",
                                             s=2, hp=2, w=W // 2),
                                cp[:].rearrange(
                                    "p (s hp h0 w w0) -> p s hp w h0 w0",
                                    s=2, hp=2, h0=2, w=W // 2, w0=2),
                                axis=mybir.AxisListType.XY,
                            )
                        else:
                            cb = sbS.tile([128, 1024], BF16, tag="cb")
                            nc.scalar.activation(cb[:], cp[:], AF.Copy)
                            s1 = sbS.tile([128, 512], BF16, tag="s1")
                            cbr = cb.rearrange("p (a w0) -> p a w0", w0=2)
                            nc.vector.tensor_max(
                                s1.rearrange("p (a o) -> p a o", o=1),
                                cbr[:, :, 0:1], cbr[:, :, 1:2],
                            )
                            s1r = s1.rearrange(
                                "p (s hp h0 w) -> p s hp h0 w",
                                s=2, hp=2, h0=2, w=W // 2)
                            nc.vector.tensor_max(
                                po.rearrange("p (s hp o w) -> p s hp o w",
                                             s=2, hp=2, o=1, w=W // 2),
                                s1r[:, :, :, 0:1, :], s1r[:, :, :, 1:2, :],
                            )
                        g += 1
                        if g in (4, 6, 8):
                            emit_gwork(g // 2 - 2)
                emit_gwork(3)

                # ---- G chain: fold G into the 64x64 conv + bias column ----
                gt_sb = sbS.tile([IC, IC], FP32, tag="Gt")
                nc.scalar.activation(gt_sb[:], g_ps[:], AF.Copy, scale=1.0 / N)
                m2_ps = psG.tile([IC, C], FP32, tag="G")
                nc.tensor.matmul(m2_ps[:], gt_sb[:], thw_sb,
                                 start=True, stop=True, skip_group_check=True)
                m2_sb = sbS.tile([IC, C], FP32, tag="m2sb")
                nc.scalar.activation(m2_sb[:], m2_ps[:], AF.Copy)
                for cpos in (0, 64):
                    nc.tensor.matmul(
                        w4_ps[cpos:cpos + C, :], m2_sb[:], wwT_sb,
                        start=False, stop=(cpos == 64),
                        tile_position=(0, cpos), skip_group_check=True,
                    )
                w4_sb = pers.tile([128, C], BF16, tag="w4sb")
                nc.scalar.activation(w4_sb[:], w4_ps[:], AF.Copy)
                v_ps = psG.tile([IC, 1], FP32, tag="G")
                nc.tensor.matmul(v_ps[:], gt_sb[:], thb_sb,
                                 start=True, stop=True, skip_group_check=True)
                v_sb = sbS.tile([IC, 1], FP32, tag="vsb")
                nc.scalar.activation(v_sb[:], v_ps[:], AF.Copy)
                b2c_ps = psG.tile([128, 1], FP32, tag="G")
                for cpos in (0, 64):
                    nc.tensor.matmul(
                        b2c_ps[cpos:cpos + C, :], wwT_sb, v_sb[:],
                        start=True, stop=False, tile_position=(0, cpos),
                        skip_group_check=True,
                    )
                    nc.tensor.matmul(
                        b2c_ps[cpos:cpos + C, :], wbr_sb[:], one_sb[:, :],
                        start=False, stop=True, tile_position=(0, cpos),
                        skip_group_check=True,
                    )
                b2c_sb = pers.tile([128, 1], FP32, tag="b2csb")
                nc.scalar.activation(b2c_sb[:], b2c_ps[:], AF.Copy)

            # ---- Phase D: final 64x64 conv over target (bf16) ----
            with tc.tile_pool(name="psD", bufs=3, space="PSUM") as psD:
                for t in range(2):
                    tt = tgt_tiles[t]
                    ot = sbO.tile([128, 2048], BF16, tag="out")
                    for i in range(4):
                        op = psD.tile([128, 512], FP32, tag="od")
                        isl = slice(i * 512, (i + 1) * 512)
                        nc.tensor.matmul(
                            op[0:C, :], w4_sb[0:C, :], tt[0:C, isl],
                            start=True, stop=True, tile_position=(0, 0),
                        )
                        nc.tensor.matmul(
                            op[C:128, :], w4_sb[C:128, :], tt[C:128, isl],
                            start=True, stop=True, tile_position=(64, 64),
                        )
                        if i % 2 == 0:
                            nc.scalar.activation(
                                ot[:, isl], op[:], AF.Identity,
                                bias=b2c_sb[:],
                            )
                        else:
                            nc.vector.tensor_scalar_add(
                                ot[:, isl], op[:], b2c_sb[:],
                            )
                    nc.sync.dma_start(
                        out=out[:, t * 2048:(t + 1) * 2048], in_=ot[:]
                    )

    nc.compile()
    return nc


def _in_maps(target, ref, ref_align, theta_w, theta_b, phi_w, phi_b,
             g_w, g_b, W_w, W_b):
    f32 = np.float32
    bf16 = ml_dtypes.bfloat16
    f8 = ml_dtypes.float8_e3m4
    wBv = np.zeros((IC, 2 * C + 1), dtype=f32)
    wBv[:, 0:C] = theta_w
    wBv[:, C:2 * C] = W_w.T
    wBv[:, 2 * C] = theta_b
    wbdv = np.zeros((128, C), dtype=f32)
    wbdv[0:C, 0:IC] = phi_w.T
    wbdv[C:128, IC:2 * IC] = g_w.T
    common = {
        "wbd": wbdv.astype(f8),
        "wB": wBv,
        "wbr": W_b.reshape(1, C).astype(f32),
        "pgbf": np.tile(np.concatenate([phi_b, g_b]), 8).reshape(1, 512).astype(bf16),
    }
    maps = []
    for core in range(8):
        b, u = core // 2, core % 2
        refsv = np.concatenate(
            [ref[b].reshape(C, N), ref_align[b].reshape(C, N)], axis=0
        ).astype(f8)
        th = target[b, :, u * (H // 2):(u + 1) * (H // 2), :].reshape(C, N // 2)
        tgtv = np.concatenate([th[:, :NT], th[:, NT:]], axis=0).astype(bf16)
        maps.append({"refs": np.ascontiguousarray(refsv),
                     "tgt": np.ascontiguousarray(tgtv), **common})
    return maps


def kernel(**inputs) -> np.ndarray:
    if "nc" not in _CACHED:
        _CACHED["nc"] = _build_program()
    nc = _CACHED["nc"]
    maps = _in_maps(**inputs)
    res = run_bass_kernel_spmd(nc, maps, list(range(8)))
    out = np.empty((B, C, H, W), dtype=np.float32)
    for core in range(8):
        o = res.results[core]["o"].astype(np.float32)  # [128, 4096] u-stacked
        half = np.concatenate([o[:C, :], o[C:, :]], axis=1)  # [64, 8192]
        b, u = core // 2, core % 2
        out[b, :, u * (H // 2):(u + 1) * (H // 2), :] = half.reshape(C, H // 2, W)
    return out


# revision 25
# speedup vs baseline: 1.4158x; 1.0672x over previous
"""NonLocal block kernel for 8 Trainium2 NeuronCores.

Algebraic restructuring: the softmax-free attention

    s = theta^T phi / N ;  y = s . g^T   (per batch)

is reassociated as y = (G/N) @ theta with G[i,j] = sum_m g[i,m] phi[j,m]
(a [32,32] matrix per batch).  Folding the surrounding 1x1 convs:

    out = (I + W_w (G/N) theta_w) @ target + (W_w (G/N) theta_b + W_b)

so after G is known the whole module is one 64x64 1x1-conv over target.

Sharding: batch b -> core pair (2b, 2b+1); each core of the pair computes
G for its batch redundantly (reads full ref/ref_align for the batch) and
produces half of the spatial output (no cross-core communication).

Precision: the G path only perturbs the output at the ~1e-3 * 3e-4 level,
far below the fp32 tolerance, so refs stream in fp8 (e3m4) and the
phi/g conv runs in fp8.  target / output are bf16 (the final conv
accumulates in fp32 PSUM); worst-case output error ~0.7% vs 2% budget.

Device layouts (per core):
  refs [128, 16384] f8e3 : rows 0:64 = ref[b] (c, h*w), 64:128 = ref_align[b]
  tgt  [128, 4096]  bf16 : target half, u-stacked: partitions 0:64 = first
                           2048 cols of the (c, 64*128) half, 64:128 = rest
  o    [128, 4096]  bf16 : output half, same u-stacking
Conv weights are block-diagonal [128 -> 64] (psum partitions 0:32 = phi,
32:64 = g); a second copy at PE column-group 64 computes the next 512
positions concurrently (partitions 64:128), so matmul pairs stream in
parallel on separate XBUSes.  2x2 maxpool = two DVE tensor_max stages
(w-pairs from PSUM, then h-pairs in bf16).  phi/g biases enter via a
PE outer-product seed of the transpose PSUM tile.
"""

import sys

for _p in ("/opt/trn_rl_repo",):
    if _p not in sys.path:
        sys.path.insert(0, _p)

import ml_dtypes
import numpy as np

import concourse.bass as bass
import concourse.mybir as mybir
from concourse import bacc
import concourse.tile as tile
from concourse.masks import make_identity
from concourse.bass_utils import run_bass_kernel_spmd

B, C, IC, H, W = 4, 64, 32, 128, 128
N = H * W            # 16384 positions per batch
NT = N // 4          # 4096 columns of u-stacked target half per core
FP32 = mybir.dt.float32
BF16 = mybir.dt.bfloat16
F8 = mybir.dt.float8e3

# refs DMA chunks (fp8 cols): small first chunks so the conv starts early
RCHUNKS = [2048, 2048, 4096, 4096, 4096]

_CACHED = {}


def _build_program() -> bass.Bass:
    nc = bacc.Bacc("TRN2", target_bir_lowering=False, debug=False)

    refs = nc.dram_tensor("refs", [128, N], F8, kind="ExternalInput")
    tgt = nc.dram_tensor("tgt", [128, NT], BF16, kind="ExternalInput")
    # all small weights packed into one wide-line DMA (64B-line tensors
    # otherwise shred into per-partition packets and finish microseconds
    # late): cols 0:16 = wbd bytes; partitions 0:32 cols 16:145 = wB;
    # partition 0 cols 145:401 = pgb bf16 bytes, cols 401:465 = W_b row
    cpk = nc.dram_tensor("cpk", [128, 465], FP32, kind="ExternalInput")
    out = nc.dram_tensor("o", [128, NT], BF16, kind="ExternalOutput")

    AF = mybir.ActivationFunctionType

    with tile.TileContext(nc) as tc:
        with (
            tc.tile_pool(name="const", bufs=1) as cpool,
            tc.tile_pool(name="refsp", bufs=3) as sbR,
            tc.tile_pool(name="small", bufs=2) as sbS,
            tc.tile_pool(name="outp", bufs=2) as sbO,
            tc.tile_pool(name="persist", bufs=1) as pers,
        ):
            # one packed const DMA first, then refs chunks, then target —
            # all on the single sync HWDGE queue in priority order
            cpk_sb = cpool.tile([128, 465], FP32, tag="cpk")
            nc.sync.dma_start(out=cpk_sb[:], in_=cpk[:])
            rts = []
            col = 0
            for k, rc in enumerate(RCHUNKS):
                rt = sbR.tile([128, rc], F8, tag=f"refs{min(k, 2)}",
                              name=f"refs{min(k, 2)}")
                nc.sync.dma_start(out=rt[:], in_=refs[:, col:col + rc])
                rts.append(rt)
                col += rc
            tgt_tiles = []
            for t in range(2):
                tt = pers.tile([128, 2048], BF16, tag=f"tgt{t}",
                               name=f"tgt{t}")
                nc.sync.dma_start(
                    out=tt[:], in_=tgt[:, t * 2048:(t + 1) * 2048])
                tgt_tiles.append(tt)

            wbd_sb = cpk_sb[:, 0:16].bitcast(F8)         # [128, 64]
            thw_sb = cpk_sb[0:IC, 16:16 + C]             # [32, 64]
            wwT_sb = cpk_sb[0:IC, 16 + C:16 + 2 * C]     # [32, 64]
            thb_sb = cpk_sb[0:IC, 16 + 2 * C:17 + 2 * C]  # [32, 1]
            pgb_sb = cpk_sb[0:1, 145:401].bitcast(BF16)  # [1, 512]
            wbr_sb = cpk_sb[0:1, 401:465]                # [1, 64]

            # device-built constants (gpsimd is otherwise idle)
            idb_sb = cpool.tile([128, 128], BF16, tag="identb")
            make_identity(nc, idb_sb[:])
            idf_sb = cpool.tile([128, 128], FP32, tag="identf")
            make_identity(nc, idf_sb[:])
            onesb_sb = cpool.tile([1, 128], BF16, tag="onesb")
            nc.gpsimd.memset(onesb_sb[:], 1.0)
            one_sb = cpool.tile([1, 1], FP32, tag="one")
            nc.gpsimd.memset(one_sb[:], 1.0)

            pooled = pers.tile([128, 2048], FP32, tag="pooled")
            phigT = pers.tile([128, 2048], BF16, tag="phigT")

            with tc.tile_pool(name="psA", bufs=2, space="PSUM") as psA, \
                 tc.tile_pool(name="psB", bufs=2, space="PSUM") as psB, \
                 tc.tile_pool(name="psG", bufs=1, space="PSUM") as psG, \
                 tc.tile_pool(name="psW", bufs=1, space="PSUM") as psW:
                g_ps = psG.tile([IC, IC], FP32, tag="G")
                w4_ps = psW.tile([128, C], FP32, tag="w4")
                # identity seed of the fused final-conv weight (I + A^T),
                # duplicated for both u-halves; group closes after G.
                for cpos in (0, 64):
                    nc.tensor.matmul(
                        w4_ps[cpos:cpos + C, :], idb_sb[0:C, 0:C],
                        idb_sb[0:C, 0:C], start=True, stop=False,
                        tile_position=(0, cpos), skip_group_check=True,
                    )

                tp_tiles = {}

                def emit_seed(t):
                    # bias seed: tp = ones(128) x pgb  (phi|g biases tiled)
                    tp = psB.tile([128, 512], FP32, tag="tp")
                    tp_tiles[t] = tp
                    nc.tensor.matmul(tp[:], onesb_sb[:], pgb_sb[:],
                                     start=True, stop=False,
                                     skip_group_check=True)

                # early PE work: warms the HAM clock gate during refs DMA
                emit_seed(0)
                emit_seed(1)

                def emit_gwork(t):
                    # transposes + psum->sbuf + G partials for tp tile t
                    # (pooled blocks 4t .. 4t+3)
                    if t not in tp_tiles:
                        emit_seed(t)
                    tp = tp_tiles.pop(t)
                    for j in range(4):
                        cidx = 4 * t + j
                        nc.tensor.matmul(
                            tp[:, j * 128:(j + 1) * 128],
                            pooled[:, cidx * 128:(cidx + 1) * 128],
                            idf_sb[:], is_transpose=True,
                            start=False, stop=True, skip_group_check=True,
                        )
                    nc.scalar.activation(
                        phigT[:, t * 512:(t + 1) * 512], tp[:], AF.Copy)
                    for j in range(4):
                        b0 = t * 512 + j * 128
                        nc.tensor.matmul(
                            g_ps[:], phigT[:, b0:b0 + IC],
                            phigT[:, b0 + IC:b0 + 2 * IC],
                            start=(t == 0 and j == 0), stop=False,
                            skip_group_check=True,
                        )
                        nc.tensor.matmul(
                            g_ps[:], phigT[:, b0 + 2 * IC:b0 + 3 * IC],
                            phigT[:, b0 + 3 * IC:b0 + 4 * IC],
                            start=False, stop=(t == 3 and j == 3),
                            skip_group_check=True,
                        )

                # ---- Phase A: fp8 conv + fused 2x2 maxpool over refs ----
                g = 0  # 2048-position group counter (0..7)
                for k, rc in enumerate(RCHUNKS):
                    rt = rts[k]
                    for j in range(rc // 2048):
                        cp = psA.tile([128, 1024], FP32, tag="conv")
                        for h in range(2):  # two 512-col slices per group
                            sl = slice(j * 2048 + h * 1024,
                                       j * 2048 + h * 1024 + 512)
                            s2 = slice(j * 2048 + h * 1024 + 512,
                                       j * 2048 + (h + 1) * 1024)
                            nc.tensor.matmul(
                                cp[0:C, h * 512:(h + 1) * 512],
                                wbd_sb[:], rt[:, sl],
                                start=True, stop=True, tile_position=(0, 0))
                            nc.tensor.matmul(
                                cp[C:128, h * 512:(h + 1) * 512],
                                wbd_sb[:], rt[:, s2],
                                start=True, stop=True, tile_position=(0, 64))
                        # 2x2 maxpool, alternating DVE-direct / ACT+DVE
                        po = pooled[:, g * 256:(g + 1) * 256]
                        if g % 2 == 0:
                            nc.vector.reduce_max(
                                po.rearrange("p (s hp w) -> p s hp w",
                                             s=2, hp=2, w=W // 2),
                                cp[:].rearrange(
                                    "p (s hp h0 w w0) -> p s hp w h0 w0",
                                    s=2, hp=2, h0=2, w=W // 2, w0=2),
                                axis=mybir.AxisListType.XY,
                            )
                        else:
                            cb = sbS.tile([128, 1024], BF16, tag="cb")
                            nc.scalar.activation(cb[:], cp[:], AF.Copy)
                            s1 = sbS.tile([128, 512], BF16, tag="s1")
                            cbr = cb.rearrange("p (a w0) -> p a w0", w0=2)
                            nc.vector.tensor_max(
                                s1.rearrange("p (a o) -> p a o", o=1),
                                cbr[:, :, 0:1], cbr[:, :, 1:2],
                            )
                            s1r = s1.rearrange(
                                "p (s hp h0 w) -> p s hp h0 w",
                                s=2, hp=2, h0=2, w=W // 2)
                            nc.vector.tensor_max(
                                po.rearrange("p (s hp o w) -> p s hp o w",
                                             s=2, hp=2, o=1, w=W // 2),
                                s1r[:, :, :, 0:1, :], s1r[:, :, :, 1:2, :],
                            )
                        g += 1
                        if g in (4, 6, 8):
                            emit_gwork(g // 2 - 2)
                emit_gwork(3)

                # ---- G chain: fold G into the 64x64 conv + bias column ----
                gt_sb = sbS.tile([IC, IC], FP32, tag="Gt")
                nc.scalar.activation(gt_sb[:], g_ps[:], AF.Copy, scale=1.0 / N)
                m2_ps = psG.tile([IC, C], FP32, tag="G")
                nc.tensor.matmul(m2_ps[:], gt_sb[:], thw_sb,
                                 start=True, stop=True, skip_group_check=True)
                m2_sb = sbS.tile([IC, C], FP32, tag="m2sb")
                nc.scalar.activation(m2_sb[:], m2_ps[:], AF.Copy)
                for cpos in (0, 64):
                    nc.tensor.matmul(
                        w4_ps[cpos:cpos + C, :], m2_sb[:], wwT_sb,
                        start=False, stop=(cpos == 64),
                        tile_position=(0, cpos), skip_group_check=True,
                    )
                w4_sb = pers.tile([128, C], BF16, tag="w4sb")
                nc.scalar.activation(w4_sb[:], w4_ps[:], AF.Copy)
                v_ps = psG.tile([IC, 1], FP32, tag="G")
                nc.tensor.matmul(v_ps[:], gt_sb[:], thb_sb,
                                 start=True, stop=True, skip_group_check=True)
                v_sb = sbS.tile([IC, 1], FP32, tag="vsb")
                nc.scalar.activation(v_sb[:], v_ps[:], AF.Copy)
                b2c_ps = psG.tile([128, 1], FP32, tag="G")
                for cpos in (0, 64):
                    nc.tensor.matmul(
                        b2c_ps[cpos:cpos + C, :], wwT_sb, v_sb[:],
                        start=True, stop=False, tile_position=(0, cpos),
                        skip_group_check=True,
                    )
                    nc.tensor.matmul(
                        b2c_ps[cpos:cpos + C, :], wbr_sb[:], one_sb[:, :],
                        start=False, stop=True, tile_position=(0, cpos),
                        skip_group_check=True,
                    )
                b2c_sb = pers.tile([128, 1], FP32, tag="b2csb")
                nc.scalar.activation(b2c_sb[:], b2c_ps[:], AF.Copy)

            # ---- Phase D: final 64x64 conv over target (bf16) ----
            with tc.tile_pool(name="psD", bufs=3, space="PSUM") as psD:
                for t in range(2):
                    tt = tgt_tiles[t]
                    ot = sbO.tile([128, 2048], BF16, tag="out")
                    for i in range(4):
                        op = psD.tile([128, 512], FP32, tag="od")
                        isl = slice(i * 512, (i + 1) * 512)
                        nc.tensor.matmul(
                            op[0:C, :], w4_sb[0:C, :], tt[0:C, isl],
                            start=True, stop=True, tile_position=(0, 0),
                        )
                        nc.tensor.matmul(
                            op[C:128, :], w4_sb[C:128, :], tt[C:128, isl],
                            start=True, stop=True, tile_position=(64, 64),
                        )
                        if i % 2 == 0:
                            nc.scalar.activation(
                                ot[:, isl], op[:], AF.Identity,
                                bias=b2c_sb[:],
                            )
                        else:
                            nc.vector.tensor_scalar_add(
                                ot[:, isl], op[:], b2c_sb[:],
                            )
                    nc.sync.dma_start(
                        out=out[:, t * 2048:(t + 1) * 2048], in_=ot[:]
                    )

    nc.compile()
    return nc


def _in_maps(target, ref, ref_align, theta_w, theta_b, phi_w, phi_b,
             g_w, g_b, W_w, W_b):
    f32 = np.float32
    bf16 = ml_dtypes.bfloat16
    f8 = ml_dtypes.float8_e3m4
    wbdv = np.zeros((128, C), dtype=f32)
    wbdv[0:C, 0:IC] = phi_w.T
    wbdv[C:128, IC:2 * IC] = g_w.T
    cpkv = np.zeros((128, 465), dtype=f32)
    cpkv[:, 0:16] = np.ascontiguousarray(wbdv.astype(f8)).view(f32)
    cpkv[0:IC, 16:16 + C] = theta_w
    cpkv[0:IC, 16 + C:16 + 2 * C] = W_w.T
    cpkv[0:IC, 16 + 2 * C] = theta_b
    pgbv = np.tile(np.concatenate([phi_b, g_b]), 8).astype(bf16)
    cpkv[0, 145:401] = pgbv.view(f32)
    cpkv[0, 401:465] = W_b
    common = {"cpk": cpkv}
    maps = []
    for core in range(8):
        b, u = core // 2, core % 2
        refsv = np.concatenate(
            [ref[b].reshape(C, N), ref_align[b].reshape(C, N)], axis=0
        ).astype(f8)
        th = target[b, :, u * (H // 2):(u + 1) * (H // 2), :].reshape(C, N // 2)
        tgtv = np.concatenate([th[:, :NT], th[:, NT:]], axis=0).astype(bf16)
        maps.append({"refs": np.ascontiguousarray(refsv),
                     "tgt": np.ascontiguousarray(tgtv), **common})
    return maps


def kernel(**inputs) -> np.ndarray:
    if "nc" not in _CACHED:
        _CACHED["nc"] = _build_program()
    nc = _CACHED["nc"]
    maps = _in_maps(**inputs)
    res = run_bass_kernel_spmd(nc, maps, list(range(8)))
    out = np.empty((B, C, H, W), dtype=np.float32)
    for core in range(8):
        o = res.results[core]["o"].astype(np.float32)  # [128, 4096] u-stacked
        half = np.concatenate([o[:C, :], o[C:, :]], axis=1)  # [64, 8192]
        b, u = core // 2, core % 2
        out[b, :, u * (H // 2):(u + 1) * (H // 2), :] = half.reshape(C, H // 2, W)
    return out


# revision 26
# speedup vs baseline: 1.4703x; 1.0385x over previous
"""NonLocal block kernel for 8 Trainium2 NeuronCores.

Algebraic restructuring: the softmax-free attention

    s = theta^T phi / N ;  y = s . g^T   (per batch)

is reassociated as y = (G/N) @ theta with G[i,j] = sum_m g[i,m] phi[j,m]
(a [32,32] matrix per batch).  Folding the surrounding 1x1 convs:

    out = (I + W_w (G/N) theta_w) @ target + (W_w (G/N) theta_b + W_b)

so after G is known the whole module is one 64x64 1x1-conv over target.

Sharding: batch b -> core pair (2b, 2b+1); each core of the pair computes
G for its batch redundantly (reads full ref/ref_align for the batch) and
produces half of the spatial output (no cross-core communication).

Precision: the G path only perturbs the output at the ~1e-3 * 3e-4 level,
far below the fp32 tolerance, so refs stream in fp8 (e3m4) and the
phi/g conv runs in fp8.  target / output are bf16 (the final conv
accumulates in fp32 PSUM); worst-case output error ~0.7% vs 2% budget.

DMA throughput here is packet-rate limited (~53M packets/s aggregate,
one packet per partition line), so every stream uses >=4KB partition
lines: the small weights are packed INTO the first refs chunk (2KB of
const bytes per partition ahead of the fp8 pixels), the target is one
8KB-line DMA, and refs stream in 6KB-line chunks.

Device layouts (per core):
  refs [128, 2048+16384] f8e3: cols 0:2048 packed consts; then rows
        0:64 = ref[b] (c, h*w), 64:128 = ref_align[b] as fp8 columns
  tgt  [128, 4096] bf16 : target half, u-stacked (partitions 0:64 =
        first 2048 cols of the (c, 64*128) half, 64:128 = rest)
  o    [128, 4096] bf16 : output half, same u-stacking
Conv weights are block-diagonal [128 -> 64] (psum partitions 0:32 = phi,
32:64 = g); a second copy at PE column-group 64 computes the next 512
positions concurrently.  The conv rhs is streamed w0-major so pooling
pairs are contiguous 256-runs: 2x2 maxpool = psum->bf16 copy (ACT/DVE)
+ two DVE tensor_max stages in bf16.  phi/g biases enter via a PE
outer-product seed of the transpose PSUM tile.
"""

import sys

for _p in ("/opt/trn_rl_repo",):
    if _p not in sys.path:
        sys.path.insert(0, _p)

import ml_dtypes
import numpy as np

import concourse.bass as bass
import concourse.mybir as mybir
from concourse import bacc
import concourse.tile as tile
from concourse.masks import make_identity
from concourse.bass_utils import run_bass_kernel_spmd

B, C, IC, H, W = 4, 64, 32, 128, 128
N = H * W            # 16384 positions per batch
NT = N // 4          # 4096 columns of u-stacked target half per core
CW = 2048            # const bytes per partition at the head of refs
FP32 = mybir.dt.float32
BF16 = mybir.dt.bfloat16
F8 = mybir.dt.float8e3

# refs DMA chunks in fp8 cols (first chunk also carries the consts)
RCHUNKS = [CW + 4096, 6144, 6144]

_CACHED = {}


def _build_program() -> bass.Bass:
    nc = bacc.Bacc("TRN2", target_bir_lowering=False, debug=False)

    refs = nc.dram_tensor("refs", [128, CW + N], F8, kind="ExternalInput")
    tgt = nc.dram_tensor("tgt", [128, NT], BF16, kind="ExternalInput")
    out = nc.dram_tensor("o", [128, NT], BF16, kind="ExternalOutput")

    AF = mybir.ActivationFunctionType

    with tile.TileContext(nc) as tc:
        with (
            tc.tile_pool(name="const", bufs=1) as cpool,
            tc.tile_pool(name="refsp", bufs=2) as sbR,
            tc.tile_pool(name="small", bufs=2) as sbS,
            tc.tile_pool(name="outp", bufs=2) as sbO,
            tc.tile_pool(name="persist", bufs=1) as pers,
        ):
            rts = []
            col = 0
            for k, rc in enumerate(RCHUNKS):
                rt = sbR.tile([128, rc], F8, tag=f"refs{k}", name=f"refs{k}")
                nc.sync.dma_start(out=rt[:], in_=refs[:, col:col + rc])
                rts.append(rt)
                col += rc
            tgt_sb = pers.tile([128, NT], BF16, tag="tgt")
            nc.sync.dma_start(out=tgt_sb[:], in_=tgt[:])

            # const views into the head of refs chunk 0
            rt0 = rts[0]
            wbd_sb = rt0[:, 0:64]                            # [128,64] f8
            m2vr_sb = rt0[0:IC, 64:324].bitcast(FP32)        # [32,65] thw|thb
            thw_sb = rt0[0:IC, 64:320].bitcast(FP32)         # [32,64]
            wwT_sb = rt0[0:IC, 324:580].bitcast(FP32)        # [32,64]
            pgb_sb = rt0[0:1, 580:1604].bitcast(BF16)        # [1,512]
            wbr_sb = rt0[0:1, 1604:1860].bitcast(FP32)       # [1,64]

            # device-built constants (gpsimd is otherwise idle)
            idb_sb = cpool.tile([128, 128], BF16, tag="identb")
            make_identity(nc, idb_sb[:])
            idf_sb = cpool.tile([128, 128], FP32, tag="identf")
            make_identity(nc, idf_sb[:])
            onesb_sb = cpool.tile([1, 128], BF16, tag="onesb")
            nc.gpsimd.memset(onesb_sb[:], 1.0)
            one_sb = cpool.tile([1, 1], FP32, tag="one")
            nc.gpsimd.memset(one_sb[:], 1.0)

            pooled = pers.tile([128, 2048], FP32, tag="pooled")
            phigT = pers.tile([128, 2048], BF16, tag="phigT")

            with tc.tile_pool(name="psA", bufs=2, space="PSUM") as psA, \
                 tc.tile_pool(name="psB", bufs=2, space="PSUM") as psB, \
                 tc.tile_pool(name="psG", bufs=1, space="PSUM") as psG, \
                 tc.tile_pool(name="psW", bufs=1, space="PSUM") as psW:
                g_ps = psG.tile([IC, IC], FP32, tag="G")
                w4_ps = psW.tile([128, C], FP32, tag="w4")
                # identity seed of the fused final-conv weight (I + A^T),
                # duplicated for both u-halves; group closes after G.
                for cpos in (0, 64):
                    nc.tensor.matmul(
                        w4_ps[cpos:cpos + C, :], idb_sb[0:C, 0:C],
                        idb_sb[0:C, 0:C], start=True, stop=False,
                        tile_position=(0, cpos), skip_group_check=True,
                    )

                tp_tiles = {}

                def emit_seed(t):
                    # bias seed: tp = ones(128) x pgb  (phi|g biases tiled)
                    tp = psB.tile([128, 512], FP32, tag="tp")
                    tp_tiles[t] = tp
                    nc.tensor.matmul(tp[:], onesb_sb[:], pgb_sb[:],
                                     start=True, stop=False,
                                     skip_group_check=True)

                # early PE work: warms the HAM clock gate during refs DMA
                emit_seed(0)
                emit_seed(1)

                def emit_gwork(t):
                    # transposes + psum->sbuf + G partials for tp tile t
                    # (pooled blocks 4t .. 4t+3)
                    if t not in tp_tiles:
                        emit_seed(t)
                    tp = tp_tiles.pop(t)
                    for j in range(4):
                        cidx = 4 * t + j
                        nc.tensor.matmul(
                            tp[:, j * 128:(j + 1) * 128],
                            pooled[:, cidx * 128:(cidx + 1) * 128],
                            idf_sb[:], is_transpose=True,
                            start=False, stop=True, skip_group_check=True,
                        )
                    nc.scalar.activation(
                        phigT[:, t * 512:(t + 1) * 512], tp[:], AF.Copy)
                    for j in range(4):
                        b0 = t * 512 + j * 128
                        nc.tensor.matmul(
                            g_ps[:], phigT[:, b0:b0 + IC],
                            phigT[:, b0 + IC:b0 + 2 * IC],
                            start=(t == 0 and j == 0), stop=False,
                            skip_group_check=True,
                        )
                        nc.tensor.matmul(
                            g_ps[:], phigT[:, b0 + 2 * IC:b0 + 3 * IC],
                            phigT[:, b0 + 3 * IC:b0 + 4 * IC],
                            start=False, stop=(t == 3 and j == 3),
                            skip_group_check=True,
                        )

                # ---- Phase A: fp8 conv + fused 2x2 maxpool over refs ----
                for g in range(8):   # 2048-position groups
                    pos = g * 2048 + CW
                    # locate chunk tile and in-tile offset
                    k, base = 0, 0
                    for kk, rc in enumerate(RCHUNKS):
                        if pos < base + rc:
                            k = kk
                            break
                        base += rc
                    rt = rts[k]
                    off = pos - base
                    cp = psA.tile([128, 1024], FP32, tag="conv")
                    for h in range(2):  # two 512-col slices per group
                        sl = slice(off + h * 1024, off + h * 1024 + 512)
                        s2 = slice(off + h * 1024 + 512,
                                   off + (h + 1) * 1024)
                        # w0-major streaming so pool pairs are contiguous
                        nc.tensor.matmul(
                            cp[0:C, h * 512:(h + 1) * 512],
                            wbd_sb,
                            rt[:, sl].rearrange("p (a w0) -> p w0 a", w0=2),
                            start=True, stop=True, tile_position=(0, 0))
                        nc.tensor.matmul(
                            cp[C:128, h * 512:(h + 1) * 512],
                            wbd_sb,
                            rt[:, s2].rearrange("p (a w0) -> p w0 a", w0=2),
                            start=True, stop=True, tile_position=(0, 64))
                    # psum -> bf16 copy (split ACT/DVE), then 2-stage pool
                    cb = sbS.tile([128, 1024], BF16, tag="cb")
                    if g % 8 in (3, 5, 7):
                        nc.vector.tensor_copy(cb[:], cp[:])
                    else:
                        nc.scalar.activation(cb[:], cp[:], AF.Copy)
                    s1 = sbS.tile([128, 512], BF16, tag="s1")
                    cbr = cb.rearrange("p (h w0 a) -> p h w0 a", h=2, w0=2,
                                       a=256)
                    nc.vector.tensor_max(
                        s1.rearrange("p (h o a) -> p h o a", h=2, o=1,
                                     a=256),
                        cbr[:, :, 0:1, :], cbr[:, :, 1:2, :],
                    )
                    s1r = s1.rearrange("p (h hp h0 w) -> p h hp h0 w",
                                       h=2, hp=2, h0=2, w=W // 2)
                    po = pooled[:, g * 256:(g + 1) * 256]
                    nc.vector.tensor_max(
                        po.rearrange("p (h hp o w) -> p h hp o w",
                                     h=2, hp=2, o=1, w=W // 2),
                        s1r[:, :, :, 0:1, :], s1r[:, :, :, 1:2, :],
                    )
                    if g in (4, 6):
                        emit_gwork(g // 2 - 2)
                emit_gwork(2)
                emit_gwork(3)

                # ---- G chain: fold G into the 64x64 conv + bias column ----
                gt_sb = sbS.tile([IC, IC], FP32, tag="Gt")
                nc.scalar.activation(gt_sb[:], g_ps[:], AF.Copy, scale=1.0 / N)
                m2v_ps = psG.tile([IC, C + 1], FP32, tag="G")
                nc.tensor.matmul(m2v_ps[:], gt_sb[:], m2vr_sb,
                                 start=True, stop=True, skip_group_check=True)
                m2v_sb = sbS.tile([IC, C + 1], FP32, tag="m2sb")
                nc.scalar.activation(m2v_sb[:], m2v_ps[:], AF.Copy)
                for cpos in (0, 64):
                    nc.tensor.matmul(
                        w4_ps[cpos:cpos + C, :], m2v_sb[:, 0:C], wwT_sb,
                        start=False, stop=(cpos == 64),
                        tile_position=(0, cpos), skip_group_check=True,
                    )
                w4_sb = pers.tile([128, C], BF16, tag="w4sb")
                nc.scalar.activation(w4_sb[:], w4_ps[:], AF.Copy)
                b2c_ps = psG.tile([128, 1], FP32, tag="G")
                for cpos in (0, 64):
                    nc.tensor.matmul(
                        b2c_ps[cpos:cpos + C, :], wwT_sb,
                        m2v_sb[:, C:C + 1],
                        start=True, stop=False, tile_position=(0, cpos),
                        skip_group_check=True,
                    )
                    nc.tensor.matmul(
                        b2c_ps[cpos:cpos + C, :], wbr_sb, one_sb[:, :],
                        start=False, stop=True, tile_position=(0, cpos),
                        skip_group_check=True,
                    )
                b2c_sb = pers.tile([128, 1], FP32, tag="b2csb")
                nc.scalar.activation(b2c_sb[:], b2c_ps[:], AF.Copy)

            # ---- Phase D: final 64x64 conv over target (bf16) ----
            with tc.tile_pool(name="psD", bufs=3, space="PSUM") as psD:
                for t in range(2):
                    ot = sbO.tile([128, 2048], BF16, tag="out")
                    for i in range(4):
                        op = psD.tile([128, 512], FP32, tag="od")
                        isl = slice(i * 512, (i + 1) * 512)
                        tsl = slice(t * 2048 + i * 512,
                                    t * 2048 + (i + 1) * 512)
                        nc.tensor.matmul(
                            op[0:C, :], w4_sb[0:C, :], tgt_sb[0:C, tsl],
                            start=True, stop=True, tile_position=(0, 0),
                        )
                        nc.tensor.matmul(
                            op[C:128, :], w4_sb[C:128, :], tgt_sb[C:128, tsl],
                            start=True, stop=True, tile_position=(64, 64),
                        )
                        if i % 2 == 0:
                            nc.scalar.activation(
                                ot[:, isl], op[:], AF.Identity,
                                bias=b2c_sb[:],
                            )
                        else:
                            nc.vector.tensor_scalar_add(
                                ot[:, isl], op[:], b2c_sb[:],
                            )
                    nc.sync.dma_start(
                        out=out[:, t * 2048:(t + 1) * 2048], in_=ot[:]
                    )

    nc.compile()
    return nc


def _in_maps(target, ref, ref_align, theta_w, theta_b, phi_w, phi_b,
             g_w, g_b, W_w, W_b):
    f32 = np.float32
    bf16 = ml_dtypes.bfloat16
    f8 = ml_dtypes.float8_e3m4
    u8 = np.uint8
    wbdv = np.zeros((128, C), dtype=f32)
    wbdv[0:C, 0:IC] = phi_w.T
    wbdv[C:128, IC:2 * IC] = g_w.T
    hdr = np.zeros((128, CW), dtype=u8)
    hdr[:, 0:64] = np.ascontiguousarray(wbdv.astype(f8)).view(u8)
    m2vr = np.concatenate([theta_w, theta_b[:, None]], axis=1).astype(f32)
    hdr[0:IC, 64:324] = np.ascontiguousarray(m2vr).view(u8)
    hdr[0:IC, 324:580] = np.ascontiguousarray(W_w.T.astype(f32)).view(u8)
    pgbv = np.tile(np.concatenate([phi_b, g_b]), 8).astype(bf16)
    hdr[0, 580:1604] = pgbv.view(u8)
    hdr[0, 1604:1860] = W_b.astype(f32).view(u8)
    maps = []
    for core in range(8):
        b, u = core // 2, core % 2
        refsv = np.empty((128, CW + N), dtype=u8)
        refsv[:, 0:CW] = hdr
        refsv[:, CW:] = np.concatenate(
            [ref[b].reshape(C, N), ref_align[b].reshape(C, N)], axis=0
        ).astype(f8).view(u8)
        th = target[b, :, u * (H // 2):(u + 1) * (H // 2), :].reshape(C, N // 2)
        tgtv = np.concatenate([th[:, :NT], th[:, NT:]], axis=0).astype(bf16)
        maps.append({"refs": refsv.view(f8),
                     "tgt": np.ascontiguousarray(tgtv)})
    return maps


def kernel(**inputs) -> np.ndarray:
    if "nc" not in _CACHED:
        _CACHED["nc"] = _build_program()
    nc = _CACHED["nc"]
    maps = _in_maps(**inputs)
    res = run_bass_kernel_spmd(nc, maps, list(range(8)))
    out = np.empty((B, C, H, W), dtype=np.float32)
    for core in range(8):
        o = res.results[core]["o"].astype(np.float32)  # [128, 4096] u-stacked
        half = np.concatenate([o[:C, :], o[C:, :]], axis=1)  # [64, 8192]
        b, u = core // 2, core % 2
        out[b, :, u * (H // 2):(u + 1) * (H // 2), :] = half.reshape(C, H // 2, W)
    return out


# revision 34
# speedup vs baseline: 1.5725x; 1.0695x over previous
"""NonLocal block kernel for 8 Trainium2 NeuronCores.

Algebraic restructuring: the softmax-free attention

    s = theta^T phi / N ;  y = s . g^T   (per batch)

is reassociated as y = (G/N) @ theta with G[i,j] = sum_m g[i,m] phi[j,m]
(a [32,32] matrix per batch).  Folding the surrounding 1x1 convs:

    out = (I + W_w (G/N) theta_w) @ target + (W_w (G/N) theta_b + W_b)

so after G is known the whole module is one 64x64 1x1-conv over target.

Sharding: batch b -> core pair (2b, 2b+1); each core of the pair computes
G for its batch redundantly (reads full ref/ref_align for the batch) and
produces half of the spatial output (no cross-core communication).

Precision: the G path only perturbs the output at the ~1e-3 * 3e-4 level,
far below the fp32 tolerance, so refs stream in fp8 (e3m4) and the
phi/g conv runs in fp8.  target / output are bf16 (the final conv
accumulates in fp32 PSUM); worst-case output error ~0.7% vs 2% budget.

DMA throughput here is packet-rate limited (~53M packets/s aggregate,
one packet per partition line), so every stream uses >=4KB partition
lines: the small weights are packed INTO the first refs chunk (2KB of
const bytes per partition ahead of the fp8 pixels), the target is one
8KB-line DMA, and refs stream in 6KB-line chunks.

Device layouts (per core):
  refs [128, 2048+16384] f8e3: cols 0:2048 packed consts; then rows
        0:64 = ref[b] (c, h*w), 64:128 = ref_align[b] as fp8 columns
  tgt  [128, 4096] bf16 : target half, u-stacked (partitions 0:64 =
        first 2048 cols of the (c, 64*128) half, 64:128 = rest)
  o    [128, 4096] bf16 : output half, same u-stacking
Conv weights are block-diagonal [128 -> 64] (psum partitions 0:32 = phi,
32:64 = g); a second copy at PE column-group 64 computes the next 512
positions concurrently.  The conv rhs is streamed w0-major so pooling
pairs are contiguous 256-runs: 2x2 maxpool = psum->bf16 copy (ACT/DVE)
+ two DVE tensor_max stages in bf16.  phi/g biases enter via a PE
outer-product seed of the transpose PSUM tile.
"""

import sys

for _p in ("/opt/trn_rl_repo",):
    if _p not in sys.path:
        sys.path.insert(0, _p)

import ml_dtypes
import numpy as np

import concourse.bass as bass
import concourse.mybir as mybir
from concourse import bacc
import concourse.tile as tile
from concourse.masks import make_identity
from concourse.bass_utils import run_bass_kernel_spmd

B, C, IC, H, W = 4, 64, 32, 128, 128
N = H * W            # 16384 positions per batch
NT = N // 4          # 4096 columns of u-stacked target half per core
CW = 2048            # const bytes per partition at the head of refs
FP32 = mybir.dt.float32
BF16 = mybir.dt.bfloat16
F8 = mybir.dt.float8e3

# refs DMA chunks in fp8 cols (first chunk also carries the consts)
RCHUNKS = [CW + 4096, 6144, 6144]

_CACHED = {}


def _build_program() -> bass.Bass:
    nc = bacc.Bacc("TRN2", target_bir_lowering=False, debug=False)

    refs = nc.dram_tensor("refs", [128, CW + N], F8, kind="ExternalInput")
    tgt = nc.dram_tensor("tgt", [128, NT], BF16, kind="ExternalInput")
    out = nc.dram_tensor("o", [128, NT], BF16, kind="ExternalOutput")

    AF = mybir.ActivationFunctionType

    with tile.TileContext(nc) as tc:
        with (
            tc.tile_pool(name="const", bufs=1) as cpool,
            tc.tile_pool(name="refsp", bufs=2) as sbR,
            tc.tile_pool(name="small", bufs=2) as sbS,
            tc.tile_pool(name="outp", bufs=2) as sbO,
            tc.tile_pool(name="persist", bufs=1) as pers,
        ):
            rts = []
            col = 0
            for k, rc in enumerate(RCHUNKS):
                rt = sbR.tile([128, rc], F8, tag=f"refs{k}", name=f"refs{k}")
                nc.sync.dma_start(out=rt[:], in_=refs[:, col:col + rc])
                rts.append(rt)
                col += rc
            tgt_sb = pers.tile([128, NT], BF16, tag="tgt")
            nc.sync.dma_start(out=tgt_sb[:], in_=tgt[:])

            # const views into the head of refs chunk 0
            rt0 = rts[0]
            wbd_sb = rt0[:, 0:64]                            # [128,64] f8
            m2vr_lo = rt0[0:IC, 64:324].bitcast(FP32)        # [32,65] thw|thb
            m2vr_hi = rt0[64:96, 64:324].bitcast(FP32)       # [32,65] dup
            wwT_sb = rt0[0:IC, 324:580].bitcast(FP32)        # [32,64]
            wbr_sb = rt0[0:1, 1604:1860].bitcast(FP32)       # [1,64]
            pgbc_sb = rt0[:, 1860:1864].bitcast(FP32)        # [128,1] biases

            # device-built constants (gpsimd is otherwise idle)
            idb_sb = cpool.tile([128, 128], BF16, tag="identb")
            make_identity(nc, idb_sb[:])
            one_sb = cpool.tile([1, 1], FP32, tag="one")
            nc.gpsimd.memset(one_sb[:], 1.0)

            pooled = pers.tile([128, 2048], BF16, tag="pooled")
            phigT = pers.tile([128, 2048], BF16, tag="phigT")

            with tc.tile_pool(name="psA", bufs=2, space="PSUM") as psA, \
                 tc.tile_pool(name="psB", bufs=2, space="PSUM") as psB, \
                 tc.tile_pool(name="psG", bufs=1, space="PSUM") as psG, \
                 tc.tile_pool(name="psW", bufs=1, space="PSUM") as psW:
                g_ps = psG.tile([IC, IC], FP32, tag="G")
                w4_ps = psW.tile([128, C], FP32, tag="w4")
                # identity seed of the fused final-conv weight (I + A^T),
                # duplicated for both u-halves; group closes after G.
                for cpos in (0, 64):
                    nc.tensor.matmul(
                        w4_ps[cpos:cpos + C, :], idb_sb[0:C, 0:C],
                        idb_sb[0:C, 0:C], start=True, stop=False,
                        tile_position=(0, cpos), skip_group_check=True,
                    )

                def emit_gwork(t):
                    # transposes + psum->sbuf + G partials for tp tile t
                    # (pooled blocks 4t .. 4t+3)
                    tp = psB.tile([128, 512], BF16, tag="tp")
                    for j in range(4):
                        cidx = 4 * t + j
                        nc.tensor.matmul(
                            tp[:, j * 128:(j + 1) * 128],
                            pooled[:, cidx * 128:(cidx + 1) * 128],
                            idb_sb[:], is_transpose=True,
                            start=True, stop=True, skip_group_check=True,
                        )
                    nc.scalar.activation(
                        phigT[:, t * 512:(t + 1) * 512], tp[:], AF.Copy)
                    for j in range(4):
                        b0 = (4 * t + j) * 128
                        nc.tensor.matmul(
                            g_ps[:], phigT[:, b0:b0 + IC],
                            phigT[:, b0 + IC:b0 + 2 * IC],
                            start=(t == 0 and j == 0), stop=False,
                            skip_group_check=True,
                        )
                        nc.tensor.matmul(
                            g_ps[:], phigT[:, b0 + 2 * IC:b0 + 3 * IC],
                            phigT[:, b0 + 3 * IC:b0 + 4 * IC],
                            start=False, stop=(t == 3 and j == 3),
                            skip_group_check=True,
                        )

                # ---- Phase A: fp8 conv + fused 2x2 maxpool over refs ----
                for g in range(8):   # 2048-position groups
                    pos = g * 2048 + CW
                    # locate chunk tile and in-tile offset
                    k, base = 0, 0
                    for kk, rc in enumerate(RCHUNKS):
                        if pos < base + rc:
                            k = kk
                            break
                        base += rc
                    rt = rts[k]
                    off = pos - base
                    cp = psA.tile([128, 1024], FP32, tag="conv")
                    for h in range(2):  # two 512-col slices per group
                        sl = slice(off + h * 1024, off + h * 1024 + 512)
                        s2 = slice(off + h * 1024 + 512,
                                   off + (h + 1) * 1024)
                        # w0-major streaming so pool pairs are contiguous
                        nc.tensor.matmul(
                            cp[0:C, h * 512:(h + 1) * 512],
                            wbd_sb,
                            rt[:, sl].rearrange("p (a w0) -> p w0 a", w0=2),
                            start=True, stop=True, tile_position=(0, 0))
                        nc.tensor.matmul(
                            cp[C:128, h * 512:(h + 1) * 512],
                            wbd_sb,
                            rt[:, s2].rearrange("p (a w0) -> p w0 a", w0=2),
                            start=True, stop=True, tile_position=(0, 64))
                    # psum -> bf16 copy with fused phi/g bias (split
                    # ACT/DVE; bias before max-pool commutes), 2-stage pool
                    cb = sbS.tile([128, 1024], BF16, tag="cb")
                    if g % 8 in (3, 5, 7):
                        nc.vector.tensor_scalar_add(cb[:], cp[:], pgbc_sb)
                    else:
                        nc.scalar.activation(cb[:], cp[:], AF.Identity,
                                             bias=pgbc_sb)
                    s1 = sbS.tile([128, 512], BF16, tag="s1")
                    cbr = cb.rearrange("p (h w0 a) -> p h w0 a", h=2, w0=2,
                                       a=256)
                    nc.vector.tensor_max(
                        s1.rearrange("p (h o a) -> p h o a", h=2, o=1,
                                     a=256),
                        cbr[:, :, 0:1, :], cbr[:, :, 1:2, :],
                    )
                    s1r = s1.rearrange("p (h hp h0 w) -> p h hp h0 w",
                                       h=2, hp=2, h0=2, w=W // 2)
                    po = pooled[:, g * 256:(g + 1) * 256]
                    nc.vector.tensor_max(
                        po.rearrange("p (h hp o w) -> p h hp o w",
                                     h=2, hp=2, o=1, w=W // 2),
                        s1r[:, :, :, 0:1, :], s1r[:, :, :, 1:2, :],
                    )
                    if g in (4, 6):
                        emit_gwork(g // 2 - 2)
                emit_gwork(2)
                emit_gwork(3)

                # ---- G chain: fold G into the 64x64 conv + bias column ----
                gt_sb = sbS.tile([IC, IC], FP32, tag="Gt")
                nc.scalar.activation(gt_sb[:], g_ps[:], AF.Copy,
                                     scale=1.0 / N)
                m2v_ps = psG.tile([IC, C + 1], FP32, tag="G")
                nc.tensor.matmul(m2v_ps[:], gt_sb[:], m2vr_lo,
                                 start=True, stop=True, skip_group_check=True)
                m2v_sb = sbS.tile([IC, C + 1], FP32, tag="m2sb")
                nc.scalar.activation(m2v_sb[:], m2v_ps[:], AF.Copy)
                for cpos in (0, 64):
                    nc.tensor.matmul(
                        w4_ps[cpos:cpos + C, :], m2v_sb[:, 0:C], wwT_sb,
                        start=False, stop=(cpos == 64),
                        tile_position=(0, cpos), skip_group_check=True,
                    )
                w4_sb = pers.tile([128, C], BF16, tag="w4sb")
                nc.scalar.activation(w4_sb[:], w4_ps[:], AF.Copy)
                b2c_ps = psG.tile([128, 1], FP32, tag="G")
                for cpos in (0, 64):
                    nc.tensor.matmul(
                        b2c_ps[cpos:cpos + C, :], wwT_sb,
                        m2v_sb[:, C:C + 1],
                        start=True, stop=False, tile_position=(0, cpos),
                        skip_group_check=True,
                    )
                    nc.tensor.matmul(
                        b2c_ps[cpos:cpos + C, :], wbr_sb, one_sb[:, :],
                        start=False, stop=True, tile_position=(0, cpos),
                        skip_group_check=True,
                    )
                b2c_sb = pers.tile([128, 1], FP32, tag="b2csb")
                nc.scalar.activation(b2c_sb[:], b2c_ps[:], AF.Copy)

            # ---- Phase D: final 64x64 conv over target (bf16) ----
            with tc.tile_pool(name="psD", bufs=3, space="PSUM") as psD:
                for t in range(2):
                    ot = sbO.tile([128, 2048], BF16, tag="out")
                    for i in range(4):
                        op = psD.tile([128, 512], FP32, tag="od")
                        isl = slice(i * 512, (i + 1) * 512)
                        tsl = slice(t * 2048 + i * 512,
                                    t * 2048 + (i + 1) * 512)
                        nc.tensor.matmul(
                            op[0:C, :], w4_sb[0:C, :], tgt_sb[0:C, tsl],
                            start=True, stop=True, tile_position=(0, 0),
                        )
                        nc.tensor.matmul(
                            op[C:128, :], w4_sb[C:128, :], tgt_sb[C:128, tsl],
                            start=True, stop=True, tile_position=(64, 64),
                        )
                        if i % 2 == 0:
                            nc.scalar.activation(
                                ot[:, isl], op[:], AF.Identity,
                                bias=b2c_sb[:],
                            )
                        else:
                            nc.vector.tensor_scalar_add(
                                ot[:, isl], op[:], b2c_sb[:],
                            )
                    nc.sync.dma_start(
                        out=out[:, t * 2048:(t + 1) * 2048], in_=ot[:]
                    )

    nc.compile()
    return nc


def _in_maps(target, ref, ref_align, theta_w, theta_b, phi_w, phi_b,
             g_w, g_b, W_w, W_b):
    f32 = np.float32
    bf16 = ml_dtypes.bfloat16
    f8 = ml_dtypes.float8_e3m4
    u8 = np.uint8
    wbdv = np.zeros((128, C), dtype=f32)
    wbdv[0:C, 0:IC] = phi_w.T
    wbdv[C:128, IC:2 * IC] = g_w.T
    hdr = np.zeros((128, CW), dtype=u8)
    hdr[:, 0:64] = np.ascontiguousarray(wbdv.astype(f8)).view(u8)
    m2vr = np.concatenate([theta_w, theta_b[:, None]], axis=1).astype(f32)
    hdr[0:IC, 64:324] = np.ascontiguousarray(m2vr).view(u8)
    hdr[64:96, 64:324] = hdr[0:IC, 64:324]
    hdr[0:IC, 324:580] = np.ascontiguousarray(W_w.T.astype(f32)).view(u8)
    hdr[0, 1604:1860] = W_b.astype(f32).view(u8)
    pgbcv = np.tile(np.concatenate([phi_b, g_b]), 2).astype(f32)
    hdr[:, 1860:1864] = pgbcv.view(u8).reshape(128, 4)
    maps = []
    for core in range(8):
        b, u = core // 2, core % 2
        refsv = np.empty((128, CW + N), dtype=u8)
        refsv[:, 0:CW] = hdr
        refsv[:, CW:] = np.concatenate(
            [ref[b].reshape(C, N), ref_align[b].reshape(C, N)], axis=0
        ).astype(f8).view(u8)
        th = target[b, :, u * (H // 2):(u + 1) * (H // 2), :].reshape(C, N // 2)
        tgtv = np.concatenate([th[:, :NT], th[:, NT:]], axis=0).astype(bf16)
        maps.append({"refs": refsv.view(f8),
                     "tgt": np.ascontiguousarray(tgtv)})
    return maps


def kernel(**inputs) -> np.ndarray:
    if "nc" not in _CACHED:
        _CACHED["nc"] = _build_program()
    nc = _CACHED["nc"]
    maps = _in_maps(**inputs)
    res = run_bass_kernel_spmd(nc, maps, list(range(8)))
    out = np.empty((B, C, H, W), dtype=np.float32)
    for core in range(8):
        o = res.results[core]["o"].astype(np.float32)  # [128, 4096] u-stacked
        half = np.concatenate([o[:C, :], o[C:, :]], axis=1)  # [64, 8192]
        b, u = core // 2, core % 2
        out[b, :, u * (H // 2):(u + 1) * (H // 2), :] = half.reshape(C, H // 2, W)
    return out
